# revision 1
# baseline (speedup 1.0000x reference)
"""Trainium2 Bass kernel for nn_CAM (channel attention module).

Reference (per batch b):
    f = x[b].reshape(N, C)                      # N = H*W = 4096, C = 512
    G = f^T f                                   # (C, C) channel gram
    A = softmax(G, axis=-1)
    out[b] = gamma * (f @ A) + x[b]

Algebraic folds used:
  * residual: x[b].reshape(N, C) == f, so out[b] = f @ (gamma * A + I);
    the residual add becomes part of the second matmul's stationary operand.
  * symmetry: G == G^T, so the gram phase only computes the upper-triangular
    128-blocks (row-block m covers columns >= 128*m, free dims 512/384/256/128)
    and the 6 lower blocks are reconstructed with cheap PE transposes.

Sharding: pure data-parallel over batch: 16 batches -> 8 cores x 2 batches.
Each core runs the identical program on its own 2-batch shard; gamma and a
512x512 identity constant are replicated.

Per-core dataflow (per batch):
  1. SWDGE DMA loads x and casts fp32 -> bf16 in flight into `fb`.
  2. Triangular gram into 4 PSUM tiles (contraction over 32 spatial chunks).
  3. PSUM -> SBUF copies, 6 fp32 PE transposes to mirror the lower blocks.
  4. Softmax over rows of G: reduce_max (DVE), Exp with -max bias (ACT,
     row sums via accum_out), reciprocal, then B = (gamma/sum)*E + I in one
     scalar_tensor_tensor (bf16).
  5. PE-transpose all 128x128 blocks of fb into `ft` (= f^T, bf16) -- placed
     after the gram so these matmuls hide the softmax latency.
  6. Matmul 2: out_tile(128n, 512) = sum_m ft[m-block]^T @ B[m] (PSUM fp32),
     copy to SBUF (DVE), DMA out.
"""

import sys

if "/opt/trn_rl_repo" not in sys.path:
    sys.path.insert(0, "/opt/trn_rl_repo")

import numpy as np
import ml_dtypes

import concourse.bacc as bacc
import concourse.mybir as mybir
import concourse.tile as tile
from concourse.alu_op_type import AluOpType
from concourse.bass_utils import run_bass_kernel_spmd

F32 = mybir.dt.float32
BF16 = mybir.dt.bfloat16
AF = mybir.ActivationFunctionType

N_CORES = 8
B_FULL, H, W, C = 16, 64, 64, 512
N = H * W                      # 4096 spatial positions per batch
B_LOC = B_FULL // N_CORES      # 2 batches per core


def build_nc(b_loc=B_LOC, n=N, c=C, num_devices=N_CORES, reps=None,
             dma_cast=True, tri_gram=True, ft_via="pe", fp8_gram=True,
             out_bf16=True, f8_on_act=False, ablate=None, lead=4,
             ftr_early=False, out_on_act_ring=False, load_grp=1):
    """Build + compile the per-core Bass program.

    reps: if set, wrap the whole body in a hardware For_i loop (timing builds).
    """
    nk = n // 128   # 128-row spatial chunks
    nm = c // 128   # 128-row channel blocks

    nc = bacc.Bacc(
        "TRN2",
        target_bir_lowering=False,
        debug=False,
        num_devices=num_devices,
    )

    x_d = nc.dram_tensor("x", [b_loc * n, c], F32, kind="ExternalInput")
    gam_d = nc.dram_tensor("gamma", [1, 1], F32, kind="ExternalInput")
    id_d = nc.dram_tensor("ident", [c, c], BF16, kind="ExternalInput")
    y_d = nc.dram_tensor("y", [b_loc * n, c], BF16 if out_bf16 else F32,
                         kind="ExternalOutput")

    with tile.TileContext(nc) as tc:
        with (
            tc.tile_pool(name="xin", bufs=6) as p_xin,
            tc.tile_pool(name="fb", bufs=2) as p_fb,
            tc.tile_pool(name="ft", bufs=2) as p_ft,
            tc.tile_pool(name="gsb", bufs=2 * nm) as p_g,
            tc.tile_pool(name="esb", bufs=2 * nm) as p_e,
            tc.tile_pool(name="bsb", bufs=2 * nm) as p_b,
            tc.tile_pool(name="stat", bufs=8 * nm) as p_stat,
            tc.tile_pool(name="outp", bufs=6) as p_out,
            tc.tile_pool(name="const", bufs=1) as p_const,
            tc.tile_pool(name="psg", bufs=2, space="PSUM") as p_psg,
            tc.tile_pool(name="pst", bufs=3, space="PSUM") as p_pst,
            tc.tile_pool(name="pso", bufs=3, space="PSUM") as p_pso,
        ):
            def body(_iv=None):
                # --- constants ---
                ident_rows = []
                for m in range(nm):
                    t = p_const.tile([128, c], BF16, tag=f"ident{m}",
                                     name=f"ident{m}")
                    nc.sync.dma_start(out=t[:, :],
                                      in_=id_d[m * 128:(m + 1) * 128, :])
                    ident_rows.append(t)
                ident128 = ident_rows[0][:, 0:128]
                idf32 = p_const.tile([128, 128], F32, tag="idf32", name="idf32")
                nc.vector.tensor_copy(idf32[:, :], ident128)

                gam1 = p_const.tile([1, 1], F32, tag="gam1", name="gam1")
                nc.sync.dma_start(out=gam1[:, :], in_=gam_d[:, :])
                gamb = p_const.tile([128, 1], F32, tag="gamb", name="gamb")
                nc.gpsimd.partition_broadcast(gamb[:, :], gam1[:, :])

                for b in range(b_loc):
                    # --- load (+cast) ---
                    fb = p_fb.tile([128, nk * c], BF16, tag="fb", name=f"fb{b}")
                    if dma_cast:
                        # ramp the first batch's groups so the first gram
                        # matmul isn't stalled behind a 2MB descriptor
                        if b == 0:
                            sizes = [1, 1, 2] + [load_grp] * ((nk - 4) // load_grp)
                        else:
                            sizes = [load_grp] * (nk // load_grp)
                        k0 = 0
                        for grp in sizes:
                            src = x_d[b * n + k0 * 128:
                                      b * n + (k0 + grp) * 128, :]
                            dst = fb[:, k0 * c:(k0 + grp) * c]
                            nc.gpsimd.dma_start(
                                out=dst.rearrange("p (j c1) -> p j c1", j=grp),
                                in_=src.rearrange("(j p) c1 -> p j c1", p=128),
                            )
                            k0 += grp
                        assert k0 == nk
                    else:
                        for k in range(nk):
                            xt = p_xin.tile([128, c], F32, tag="xin",
                                            name=f"x{b}_{k}")
                            nc.sync.dma_start(
                                out=xt[:, :],
                                in_=x_d[b * n + k * 128: b * n + (k + 1) * 128, :],
                            )
                            nc.vector.tensor_copy(fb[:, k * c:(k + 1) * c],
                                                  xt[:, :])

                    if ablate == "loads":
                        continue
                    # --- gram (triangular, row-at-a-time) + softmax ---
                    # m-outer: finish one G row-block, then immediately copy
                    # it out, mirror its lower blocks from earlier rows, and
                    # run its softmax while the next row's matmuls stream.
                    if fp8_gram:
                        f8 = p_fb.tile([128, nk * c], mybir.dt.float8e4,
                                       tag="f8", name=f"f8{b}", bufs=1)
                        for k in range(nk):
                            if f8_on_act:
                                nc.scalar.copy(f8[:, k * c:(k + 1) * c],
                                               fb[:, k * c:(k + 1) * c])
                            else:
                                nc.vector.tensor_copy(f8[:, k * c:(k + 1) * c],
                                                      fb[:, k * c:(k + 1) * c])

                    ft = p_ft.tile([128, nm, n], BF16, tag="ft", name=f"ft{b}")

                    def ftr(k):
                        fbk = fb[:, k * c:(k + 1) * c]
                        if ft_via == "dma":
                            nc.sync.dma_start_transpose(
                                ft[:, :, k * 128:(k + 1) * 128], fbk,
                            )
                            return
                        ps_t = p_pst.tile([128, c], BF16, tag="pst",
                                          name=f"pst{b}_{k}")
                        for m in range(nm):
                            nc.tensor.transpose(
                                ps_t[:, m * 128:(m + 1) * 128],
                                fbk[:, m * 128:(m + 1) * 128],
                                ident128,
                            )
                        nc.scalar.copy(
                            ft[:, :, k * 128:(k + 1) * 128],
                            ps_t[:, :].rearrange("p (m j) -> p m j", m=nm),
                        )

                    if ftr_early:
                        # transposes only need their own chunk -- run them in
                        # the load window where gram rows can't complete yet
                        for k in range(nk):
                            ftr(k)

                    g_sb = []
                    b_rows = []
                    for m in range(nm):
                        lo = m * 128 if tri_gram else 0
                        ps = p_psg.tile([128, c], F32, tag="psg",
                                        name=f"psg{b}_{m}")
                        if fp8_gram:
                            for kp in range(nk // 2):
                                sl = (f8[:, 2 * kp * c:(2 * kp + 2) * c]
                                      .rearrange("p (o c1) -> p o c1", o=2))
                                nc.tensor.matmul(
                                    ps[:, lo:c],
                                    sl[:, :, m * 128:(m + 1) * 128],
                                    sl[:, :, lo:c],
                                    start=(kp == 0),
                                    stop=(kp == nk // 2 - 1),
                                    perf_mode=mybir.MatmulPerfMode.DoubleRow,
                                )
                        else:
                            for k in range(nk):
                                fbk = fb[:, k * c:(k + 1) * c]
                                nc.tensor.matmul(
                                    ps[:, lo:c],
                                    fbk[:, m * 128:(m + 1) * 128],
                                    fbk[:, lo:c],
                                    start=(k == 0),
                                    stop=(k == nk - 1),
                                )
                        t_g = p_g.tile([128, c], F32, tag="gsb",
                                       name=f"g{b}_{m}")
                        nc.vector.tensor_copy(t_g[:, lo:c], ps[:, lo:c])
                        if tri_gram:
                            for d in range(m):
                                tp = p_pso.tile([128, 128], F32, tag="pso",
                                                name=f"gt{b}_{m}_{d}")
                                nc.tensor.transpose(
                                    tp[:, :],
                                    g_sb[d][:, m * 128:(m + 1) * 128],
                                    idf32[:, :],
                                )
                                nc.vector.tensor_copy(
                                    t_g[:, d * 128:(d + 1) * 128], tp[:, :])
                        g_sb.append(t_g)

                        nmax = p_stat.tile([128, 1], F32, tag="nmax",
                                           name=f"nmax{b}_{m}")
                        nc.vector.reduce_max(
                            nmax[:, :], t_g[:, :], axis=mybir.AxisListType.X,
                            negate=True,
                        )
                        e_sb = p_e.tile([128, c], BF16, tag="esb",
                                        name=f"e{b}_{m}")
                        esum = p_stat.tile([128, 1], F32, tag="esum",
                                           name=f"esum{b}_{m}")
                        nc.scalar.activation(
                            e_sb[:, :], t_g[:, :], AF.Exp,
                            bias=nmax[:, :], scale=1.0, accum_out=esum[:, :],
                        )
                        rec = p_stat.tile([128, 1], F32, tag="rec",
                                          name=f"rec{b}_{m}")
                        nc.vector.reciprocal(rec[:, :], esum[:, :])
                        sc = p_stat.tile([128, 1], F32, tag="sc",
                                         name=f"sc{b}_{m}")
                        nc.vector.tensor_tensor(
                            sc[:, :], rec[:, :], gamb[:, :], op=AluOpType.mult,
                        )
                        b_sb = p_b.tile([128, c], BF16, tag="bsb",
                                        name=f"bmat{b}_{m}")
                        nc.vector.scalar_tensor_tensor(
                            b_sb[:, :], e_sb[:, :], sc[:, :],
                            ident_rows[m][:, :],
                            op0=AluOpType.mult, op1=AluOpType.add,
                        )
                        b_rows.append(b_sb)

                    if ablate == "gram":
                        continue
                    # --- out = f @ B, interleaved with the f-transposes:
                    # mm2 tile t only needs the transpose of chunk t; running
                    # the transposes LEAD chunks ahead keeps the PE warm while
                    # giving the ACT psum->sbuf copy time to land.
                    LEAD = lead
                    if not ftr_early:
                        for t in range(min(LEAD, nk)):
                            ftr(t)
                    for t in range(nk):
                        if not ftr_early and t + LEAD < nk:
                            ftr(t + LEAD)
                        ps_o = p_pso.tile([128, c], F32, tag="pso",
                                          name=f"pso{b}_{t}")
                        for m in range(nm):
                            nc.tensor.matmul(
                                ps_o[:, :],
                                ft[:, m, t * 128:(t + 1) * 128],
                                b_rows[m][:, :],
                                start=(m == 0),
                                stop=(m == nm - 1),
                            )
                        o_sb = p_out.tile([128, c],
                                          BF16 if out_bf16 else F32,
                                          tag="outp", name=f"o{b}_{t}")
                        if t % 2 == 0:
                            nc.vector.tensor_copy(o_sb[:, :], ps_o[:, :])
                        else:
                            nc.scalar.copy(o_sb[:, :], ps_o[:, :])
                        (nc.scalar if out_on_act_ring else nc.sync).dma_start(
                            out=y_d[b * n + t * 128: b * n + (t + 1) * 128, :],
                            in_=o_sb[:, :],
                        )

            if reps is None:
                body()
            else:
                with tc.For_i(0, reps, 1,
                              hint_engines=(mybir.EngineType.PE,
                                            mybir.EngineType.DVE,
                                            mybir.EngineType.Activation)) as iv:
                    body(iv)

    nc.compile()
    return nc


_NC_CACHE = {}


def _get_nc():
    if "full" not in _NC_CACHE:
        _NC_CACHE["full"] = build_nc()
    return _NC_CACHE["full"]


def make_in_maps(inputs_np, gamma_np):
    """Shard full inputs into per-core in_maps."""
    x = np.ascontiguousarray(
        np.asarray(inputs_np, dtype=np.float32).reshape(B_FULL, N, C)
    )
    gam = np.asarray(gamma_np, dtype=np.float32).reshape(1, 1)
    ident = np.eye(C, dtype=np.float32).astype(ml_dtypes.bfloat16)
    in_maps = []
    for core in range(N_CORES):
        xs = x[core * B_LOC:(core + 1) * B_LOC].reshape(B_LOC * N, C)
        in_maps.append({
            "x": np.ascontiguousarray(xs),
            "gamma": gam,
            "ident": ident,
        })
    return in_maps


def kernel(inputs, gamma):
    nc = _get_nc()
    in_maps = make_in_maps(inputs, gamma)
    res = run_bass_kernel_spmd(nc, in_maps, core_ids=list(range(N_CORES)))
    outs = [np.asarray(res.results[c]["y"], dtype=np.float32)
            .reshape(B_LOC, N, C) for c in range(N_CORES)]
    y = np.concatenate(outs, axis=0).reshape(B_FULL, H, W, C)
    return y.astype(np.float32)



# revision 6
# speedup vs baseline: 1.0664x; 1.0664x over previous
"""Trainium2 Bass kernel for nn_CAM (channel attention module).

Reference (per batch b):
    f = x[b].reshape(N, C)                      # N = H*W = 4096, C = 512
    G = f^T f                                   # (C, C) channel gram
    A = softmax(G, axis=-1)
    out[b] = gamma * (f @ A) + x[b]

Algebraic folds:
  * residual: out[b] = f @ (gamma * A + I) -- residual add folded into the
    second matmul's moving operand.
  * symmetry: G == G^T, so only upper-triangular 128-blocks are computed
    (free dims 512/384/256/128); the 6 lower blocks are PE-transposed back.

Layout: n rows are interleaved 2-per-partition (row 256k + 2p + j lives on
partition p, slice j of chunk k).  This makes load descriptors 4KB and store
descriptors 2KB contiguous (vs 2KB/1KB non-interleaved), which measured
~25% faster stores under full 8-core HBM contention.  The gram is invariant
to the n-permutation; ft/MM2/store all use the same ordering consistently.

Schedule (per core, 2 batches, streaming):
  - chunk k of batch b arrives (SWDGE cast fp32->bf16, 16 chunks/batch)
  - DVE casts it to fp8; PE transposes it into ft (f^T) and accumulates the
    triangular gram into 4 parallel PSUM banks (fp8 DoubleRow, 256-row
    contraction per chunk)
  - after the last chunk: G rows copy to SBUF (bf16), lower blocks are
    mirrored by 6 PE transposes, then per-row softmax (DVE max / ACT exp
    with accum / DVE reciprocal+scale) produces B = gamma/s * E + I (bf16)
  - MM2: out rows = ft_chunk^T @ B accumulated over the 4 channel blocks in
    PSUM; batch 0's MM2 groups are interleaved 1:1 with batch 1's chunk
    work so the PE cadence (2.4us/pair) matches the DMA cadence.
  - PSUM->SBUF copies are split ACT:DVE ~2:1; stores are HWDGE (sync).

Sharding: pure data-parallel over batch: 16 batches -> 8 cores x 2.
"""

import sys

if "/opt/trn_rl_repo" not in sys.path:
    sys.path.insert(0, "/opt/trn_rl_repo")

import numpy as np
import ml_dtypes

import concourse.bacc as bacc
import concourse.mybir as mybir
import concourse.tile as tile
from concourse.alu_op_type import AluOpType
from concourse.bass_utils import run_bass_kernel_spmd

F32 = mybir.dt.float32
BF16 = mybir.dt.bfloat16
FP8 = mybir.dt.float8e4
AF = mybir.ActivationFunctionType

N_CORES = 8
B_FULL, H, W, C = 16, 64, 64, 512
N = H * W                      # 4096 spatial positions per batch
B_LOC = B_FULL // N_CORES      # 2 batches per core
NM = C // 128                  # 4 channel blocks
NKC = N // 256                 # 16 interleaved 256-row chunks per batch


def build_nc(b_loc=B_LOC, n=N, c=C, num_devices=N_CORES, reps=None,
             ablate=None, staggered=True, act_share=3, **_legacy):
    """Build + compile the per-core Bass program.

    reps: if set, wrap the body in a hardware For_i loop (timing builds).
    act_share: of every act_share psum->sbuf copies, 1 goes to DVE, the
        rest to ACT.
    """
    nkc = n // 256   # interleaved 256-row chunks
    nm = c // 128

    nc = bacc.Bacc(
        "TRN2",
        target_bir_lowering=False,
        debug=False,
        num_devices=num_devices,
    )

    x_d = nc.dram_tensor("x", [b_loc * n, c], F32, kind="ExternalInput")
    gam_d = nc.dram_tensor("gamma", [1, 1], F32, kind="ExternalInput")
    id_d = nc.dram_tensor("ident", [c, c], BF16, kind="ExternalInput")
    y_d = nc.dram_tensor("y", [b_loc * n, c], BF16, kind="ExternalOutput")

    with tile.TileContext(nc) as tc:
        with (
            tc.tile_pool(name="fbc", bufs=6) as p_fb,      # bf16 chunk staging
            tc.tile_pool(name="f8c", bufs=6) as p_f8,      # fp8 chunk staging
            tc.tile_pool(name="ft", bufs=2) as p_ft,       # f^T per batch
            tc.tile_pool(name="gsb", bufs=2 * nm) as p_g,
            tc.tile_pool(name="esb", bufs=2 * nm) as p_e,
            tc.tile_pool(name="bsb", bufs=2 * nm) as p_b,
            tc.tile_pool(name="stat", bufs=8 * nm) as p_stat,
            tc.tile_pool(name="outp", bufs=6) as p_out,
            tc.tile_pool(name="const", bufs=1) as p_const,
            tc.tile_pool(name="psg", bufs=1, space="PSUM") as p_psg,
            tc.tile_pool(name="pst", bufs=2, space="PSUM") as p_pst,
            tc.tile_pool(name="pso", bufs=2, space="PSUM") as p_pso,
        ):
            # --- constants (outside the timing loop) ---
            ident_rows = []
            for m in range(nm):
                t = p_const.tile([128, c], BF16, tag=f"ident{m}",
                                 name=f"ident{m}")
                nc.sync.dma_start(out=t[:, :],
                                  in_=id_d[m * 128:(m + 1) * 128, :])
                ident_rows.append(t)
            ident128 = ident_rows[0][:, 0:128]
            gam1 = p_const.tile([1, 1], F32, tag="gam1", name="gam1")
            nc.sync.dma_start(out=gam1[:, :], in_=gam_d[:, :])
            gamb = p_const.tile([128, 1], F32, tag="gamb", name="gamb")
            nc.gpsimd.partition_broadcast(gamb[:, :], gam1[:, :])

            copy_ctr = [0]

            def cp_engine():
                """Rotate psum->sbuf copies: 1 in act_share goes to DVE."""
                i = copy_ctr[0]
                copy_ctr[0] += 1
                if i % act_share == 0:
                    return nc.vector.tensor_copy
                return nc.scalar.copy

            def load_chunk(b, k):
                fbk = p_fb.tile([128, 2, c], BF16, tag="fbc",
                                name=f"fb{b}_{k}")
                base = b * n + k * 256
                nc.gpsimd.dma_start(
                    out=fbk[:, :, :],
                    in_=x_d[base:base + 256, :]
                        .rearrange("(p j) c1 -> p j c1", p=128),
                )
                return fbk

            def chunk_work(b, k, fbk, ft, psg_rows):
                """f8 cast + ft transposes + gram accumulation for chunk k."""
                f8k = p_f8.tile([128, 2, c], FP8, tag="f8c",
                                name=f"f8{b}_{k}")
                nc.vector.tensor_copy(f8k[:, :, :], fbk[:, :, :])
                # gram: one DoubleRow matmul per row-block (256-contraction)
                for m in range(nm):
                    lo = m * 128
                    nc.tensor.matmul(
                        psg_rows[m][:, lo:c],
                        f8k[:, :, m * 128:(m + 1) * 128],
                        f8k[:, :, lo:c],
                        start=(k == 0),
                        stop=(k == nkc - 1),
                        perf_mode=mybir.MatmulPerfMode.DoubleRow,
                    )
                # transposes: 4 per j-slice into ft columns
                for j in range(2):
                    ps_t = p_pst.tile([128, c], BF16, tag="pst",
                                      name=f"pst{b}_{k}_{j}")
                    for m in range(nm):
                        nc.tensor.transpose(
                            ps_t[:, m * 128:(m + 1) * 128],
                            fbk[:, j, m * 128:(m + 1) * 128],
                            ident128,
                        )
                    cp_engine()(
                        ft[:, :, k * 256 + j * 128: k * 256 + (j + 1) * 128],
                        ps_t[:, :].rearrange("p (m f) -> p m f", m=nm),
                    )

            def gram_finish(b, psg_rows):
                """PSUM G -> SBUF (bf16), mirror lower blocks, softmax -> B."""
                g_sb = []
                for m in range(nm):
                    lo = m * 128
                    t_g = p_g.tile([128, c], BF16, tag="gsb", name=f"g{b}_{m}")
                    nc.vector.tensor_copy(t_g[:, lo:c], psg_rows[m][:, lo:c])
                    g_sb.append(t_g)
                for m in range(1, nm):
                    for d in range(m):
                        tp = p_pst.tile([128, c], BF16, tag="pst",
                                        name=f"gt{b}_{m}_{d}")
                        nc.tensor.transpose(
                            tp[:, 0:128], g_sb[d][:, m * 128:(m + 1) * 128],
                            ident128,
                        )
                        nc.vector.tensor_copy(
                            g_sb[m][:, d * 128:(d + 1) * 128], tp[:, 0:128])
                b_rows = []
                for m in range(nm):
                    t_g = g_sb[m]
                    nmax = p_stat.tile([128, 1], F32, tag="nmax",
                                       name=f"nmax{b}_{m}")
                    nc.vector.reduce_max(
                        nmax[:, :], t_g[:, :], axis=mybir.AxisListType.X,
                        negate=True,
                    )
                    e_sb = p_e.tile([128, c], BF16, tag="esb",
                                    name=f"e{b}_{m}")
                    esum = p_stat.tile([128, 1], F32, tag="esum",
                                       name=f"esum{b}_{m}")
                    nc.scalar.activation(
                        e_sb[:, :], t_g[:, :], AF.Exp,
                        bias=nmax[:, :], scale=1.0, accum_out=esum[:, :],
                    )
                    rec = p_stat.tile([128, 1], F32, tag="rec",
                                      name=f"rec{b}_{m}")
                    nc.vector.reciprocal(rec[:, :], esum[:, :])
                    sc = p_stat.tile([128, 1], F32, tag="sc",
                                     name=f"sc{b}_{m}")
                    nc.vector.tensor_tensor(
                        sc[:, :], rec[:, :], gamb[:, :], op=AluOpType.mult,
                    )
                    b_sb = p_b.tile([128, c], BF16, tag="bsb",
                                    name=f"bmat{b}_{m}")
                    nc.vector.scalar_tensor_tensor(
                        b_sb[:, :], e_sb[:, :], sc[:, :],
                        ident_rows[m][:, :],
                        op0=AluOpType.mult, op1=AluOpType.add,
                    )
                    b_rows.append(b_sb)
                return b_rows

            def mm2_group(b, k, ft, b_rows):
                """Second matmul + store for one 256-row group."""
                o_sb = p_out.tile([128, 2, c], BF16, tag="outp",
                                  name=f"o{b}_{k}")
                for j in range(2):
                    ps_o = p_pso.tile([128, c], F32, tag="pso",
                                      name=f"pso{b}_{k}_{j}")
                    for m in range(nm):
                        nc.tensor.matmul(
                            ps_o[:, :],
                            ft[:, m, k * 256 + j * 128:
                               k * 256 + (j + 1) * 128],
                            b_rows[m][:, :],
                            start=(m == 0),
                            stop=(m == nm - 1),
                        )
                    cp_engine()(o_sb[:, j, :], ps_o[:, :])
                base = b * n + k * 256
                nc.sync.dma_start(
                    out=y_d[base:base + 256, :]
                        .rearrange("(p j) c1 -> p j c1", p=128),
                    in_=o_sb[:, :, :],
                )

            def body(_iv=None):
                # ---- batch 0 stream ----
                ft0 = p_ft.tile([128, nm, n], BF16, tag="ft", name="ft0")
                psg0 = [p_psg.tile([128, c], F32, tag=f"psg{m}",
                                   name=f"psg0_{m}") for m in range(nm)]
                fb0 = [load_chunk(0, k) for k in range(nkc)]
                if ablate == "loads":
                    for k in range(nkc):
                        # keep a reader so tiles are consumed
                        f8k = p_f8.tile([128, 2, c], FP8, tag="f8c",
                                        name=f"f8d0_{k}")
                        nc.vector.tensor_copy(f8k[:, :, :], fb0[k][:, :, :])
                    return
                for k in range(nkc):
                    chunk_work(0, k, fb0[k], ft0, psg0)
                b_rows0 = gram_finish(0, psg0)
                if ablate == "gram":
                    return

                # ---- batch 1 stream interleaved with batch 0 MM2 ----
                ft1 = p_ft.tile([128, nm, n], BF16, tag="ft", name="ft1")
                psg1 = [p_psg.tile([128, c], F32, tag=f"psg{m}",
                                   name=f"psg1_{m}") for m in range(nm)]
                fb1 = [load_chunk(1, k) for k in range(nkc)]
                for k in range(nkc):
                    mm2_group(0, k, ft0, b_rows0)
                    chunk_work(1, k, fb1[k], ft1, psg1)
                b_rows1 = gram_finish(1, psg1)
                for k in range(nkc):
                    mm2_group(1, k, ft1, b_rows1)

            if reps is None:
                body()
            else:
                with tc.For_i(0, reps, 1,
                              staggered_reset=staggered,
                              hint_engines=(mybir.EngineType.PE,
                                            mybir.EngineType.DVE,
                                            mybir.EngineType.Activation,
                                            mybir.EngineType.Pool,
                                            mybir.EngineType.SP)) as iv:
                    body(iv)

    nc.compile()
    return nc


_NC_CACHE = {}


def _get_nc():
    if "full" not in _NC_CACHE:
        _NC_CACHE["full"] = build_nc()
    return _NC_CACHE["full"]


def make_in_maps(inputs_np, gamma_np):
    """Shard full inputs into per-core in_maps."""
    x = np.ascontiguousarray(
        np.asarray(inputs_np, dtype=np.float32).reshape(B_FULL, N, C)
    )
    gam = np.asarray(gamma_np, dtype=np.float32).reshape(1, 1)
    ident = np.eye(C, dtype=np.float32).astype(ml_dtypes.bfloat16)
    in_maps = []
    for core in range(N_CORES):
        xs = x[core * B_LOC:(core + 1) * B_LOC].reshape(B_LOC * N, C)
        in_maps.append({
            "x": np.ascontiguousarray(xs),
            "gamma": gam,
            "ident": ident,
        })
    return in_maps


def kernel(inputs, gamma):
    nc = _get_nc()
    in_maps = make_in_maps(inputs, gamma)
    res = run_bass_kernel_spmd(nc, in_maps, core_ids=list(range(N_CORES)))
    outs = [np.asarray(res.results[c]["y"], dtype=np.float32)
            .reshape(B_LOC, N, C) for c in range(N_CORES)]
    y = np.concatenate(outs, axis=0).reshape(B_FULL, H, W, C)
    return y.astype(np.float32)


# revision 17
# speedup vs baseline: 1.2250x; 1.1487x over previous
"""Trainium2 Bass kernel for nn_CAM (channel attention module).

Reference (per batch b):
    f = x[b].reshape(N, C)                      # N = H*W = 4096, C = 512
    G = f^T f                                   # (C, C) channel gram
    A = softmax(G, axis=-1)
    out[b] = gamma * (f @ A) + x[b]

Algebraic folds:
  * residual: out[b] = f @ (gamma * A + I) -- residual add folded into the
    second matmul's moving operand.
  * symmetry: G == G^T, so only upper-triangular 128-blocks are computed
    (free dims 512/384/256/128); the 6 lower blocks are PE-transposed back.

Layout: n rows are interleaved 2-per-partition (row 256k + 2p + j lives on
partition p, slice j of chunk k).  This makes load descriptors 4KB and store
descriptors 2KB contiguous (vs 2KB/1KB non-interleaved), which measured
~25% faster stores under full 8-core HBM contention.  The gram is invariant
to the n-permutation; ft/MM2/store all use the same ordering consistently.

Schedule (per core, 2 batches, streaming):
  - chunk k of batch b arrives (SWDGE cast fp32->bf16, 16 chunks/batch)
  - DVE casts it to fp8; PE transposes it into ft (f^T) and accumulates the
    triangular gram into 4 parallel PSUM banks (fp8 DoubleRow, 256-row
    contraction per chunk)
  - after the last chunk: G rows copy to SBUF (bf16), lower blocks are
    mirrored by 6 PE transposes, then per-row softmax (DVE max / ACT exp
    with accum / DVE reciprocal+scale) produces B = gamma/s * E + I (bf16)
  - MM2: out rows = ft_chunk^T @ B accumulated over the 4 channel blocks in
    PSUM; batch 0's MM2 groups are interleaved 1:1 with batch 1's chunk
    work so the PE cadence (2.4us/pair) matches the DMA cadence.
  - PSUM->SBUF copies are split ACT:DVE ~2:1; stores are HWDGE (sync).

Sharding: pure data-parallel over batch: 16 batches -> 8 cores x 2.
"""

import sys

if "/opt/trn_rl_repo" not in sys.path:
    sys.path.insert(0, "/opt/trn_rl_repo")

import numpy as np
import ml_dtypes

import concourse.bacc as bacc
import concourse.mybir as mybir
import concourse.tile as tile
from concourse.alu_op_type import AluOpType
from concourse.bass_utils import run_bass_kernel_spmd

F32 = mybir.dt.float32
BF16 = mybir.dt.bfloat16
FP8 = mybir.dt.float8e4
AF = mybir.ActivationFunctionType

N_CORES = 8
B_FULL, H, W, C = 16, 64, 64, 512
N = H * W                      # 4096 spatial positions per batch
B_LOC = B_FULL // N_CORES      # 2 batches per core
NM = C // 128                  # 4 channel blocks
NKC = N // 256                 # 16 interleaved 256-row chunks per batch


def build_nc(b_loc=B_LOC, n=N, c=C, num_devices=N_CORES, reps=None,
             ablate=None, staggered=True, act_share=3, cw_first=True,
             **_legacy):
    """Build + compile the per-core Bass program.

    reps: if set, wrap the body in a hardware For_i loop (timing builds).
    act_share: of every act_share psum->sbuf copies, 1 goes to DVE, the
        rest to ACT.
    """
    nkc = n // 256   # interleaved 256-row chunks
    nm = c // 128

    nc = bacc.Bacc(
        "TRN2",
        target_bir_lowering=False,
        debug=False,
        num_devices=num_devices,
    )

    x_d = nc.dram_tensor("x", [b_loc * n, c], F32, kind="ExternalInput")
    gam_d = nc.dram_tensor("gamma", [1, 1], F32, kind="ExternalInput")
    id_d = nc.dram_tensor("ident", [c, c], BF16, kind="ExternalInput")
    y_d = nc.dram_tensor("y", [b_loc * n, c], BF16, kind="ExternalOutput")

    with tile.TileContext(nc) as tc:
        with (
            tc.tile_pool(name="fbc", bufs=11) as p_fb,     # bf16 chunk staging
            tc.tile_pool(name="f8c", bufs=8) as p_f8,      # fp8 chunk staging
            tc.tile_pool(name="ft", bufs=2) as p_ft,       # f^T per batch
            tc.tile_pool(name="gsb", bufs=2 * nm) as p_g,
            tc.tile_pool(name="esb", bufs=2 * nm) as p_e,
            tc.tile_pool(name="bsb", bufs=2 * nm) as p_b,
            tc.tile_pool(name="stat", bufs=8 * nm) as p_stat,
            tc.tile_pool(name="outp", bufs=6) as p_out,
            tc.tile_pool(name="const", bufs=1) as p_const,
            tc.tile_pool(name="psg", bufs=1, space="PSUM") as p_psg,
            tc.tile_pool(name="pst", bufs=2, space="PSUM") as p_pst,
            tc.tile_pool(name="pso", bufs=2, space="PSUM") as p_pso,
        ):
            # --- constants (outside the timing loop) ---
            ident_rows = []
            for m in range(nm):
                t = p_const.tile([128, c], BF16, tag=f"ident{m}",
                                 name=f"ident{m}")
                nc.sync.dma_start(out=t[:, :],
                                  in_=id_d[m * 128:(m + 1) * 128, :])
                ident_rows.append(t)
            ident128 = ident_rows[0][:, 0:128]
            gam1 = p_const.tile([1, 1], F32, tag="gam1", name="gam1")
            nc.sync.dma_start(out=gam1[:, :], in_=gam_d[:, :])
            gamb = p_const.tile([128, 1], F32, tag="gamb", name="gamb")
            nc.gpsimd.partition_broadcast(gamb[:, :], gam1[:, :])

            copy_ctr = [0]

            def cp_engine():
                """Rotate psum->sbuf copies: 1 in act_share goes to DVE."""
                i = copy_ctr[0]
                copy_ctr[0] += 1
                if i % act_share == 0:
                    return nc.vector.tensor_copy
                return nc.scalar.copy

            def load_chunk(b, k):
                fbk = p_fb.tile([128, 2, c], BF16, tag="fbc",
                                name=f"fb{b}_{k}")
                base = b * n + k * 256
                nc.gpsimd.dma_start(
                    out=fbk[:, :, :],
                    in_=x_d[base:base + 256, :]
                        .rearrange("(p j) c1 -> p j c1", p=128),
                )
                return fbk

            def gram_part(b, k, fbk, psg_rows):
                """f8 cast + gram accumulation for chunk k."""
                f8k = p_f8.tile([128, 2, c], FP8, tag="f8c",
                                name=f"f8{b}_{k}")
                nc.vector.tensor_copy(f8k[:, :, :], fbk[:, :, :])
                # gram: one DoubleRow matmul per row-block (256-contraction)
                for m in range(nm if ablate != "nogram" else 0):
                    lo = m * 128
                    nc.tensor.matmul(
                        psg_rows[m][:, lo:c],
                        f8k[:, :, m * 128:(m + 1) * 128],
                        f8k[:, :, lo:c],
                        start=(k == 0),
                        stop=(k == nkc - 1),
                        perf_mode=mybir.MatmulPerfMode.DoubleRow,
                    )

            def tpose_part(b, k, fbk, ft):
                """8 transposes per chunk into one staging tile, one copy."""
                ps_t = p_pst.tile([128, 2, c], BF16, tag="pst",
                                  name=f"pst{b}_{k}")
                for j in range(2):
                    for m in range(nm):
                        nc.tensor.transpose(
                            ps_t[:, j, m * 128:(m + 1) * 128],
                            fbk[:, j, m * 128:(m + 1) * 128],
                            ident128,
                        )
                cp_engine()(
                    ft[:, :, k * 256: (k + 1) * 256]
                        .rearrange("p m (j f) -> p j m f", j=2),
                    ps_t[:, :, :].rearrange("p j (m f) -> p j m f", m=nm),
                )

            def chunk_work(b, k, fbk, ft, psg_rows):
                gram_part(b, k, fbk, psg_rows)
                tpose_part(b, k, fbk, ft)

            def gram_finish(b, psg_rows):
                """PSUM G -> SBUF (bf16), mirror lower blocks, softmax -> B.

                Progressive: row m's softmax is emitted as soon as its copy
                and mirror blocks are available, so b_rows[0] (and MM2) can
                start while later rows are still being assembled.
                """
                g_sb = []
                b_rows = []

                def softmax_row(m):
                    t_g = g_sb[m]
                    nmax = p_stat.tile([128, 1], F32, tag="nmax",
                                       name=f"nmax{b}_{m}")
                    nc.vector.reduce_max(
                        nmax[:, :], t_g[:, :], axis=mybir.AxisListType.X,
                        negate=True,
                    )
                    e_sb = p_e.tile([128, c], BF16, tag="esb",
                                    name=f"e{b}_{m}")
                    esum = p_stat.tile([128, 1], F32, tag="esum",
                                       name=f"esum{b}_{m}")
                    nc.scalar.activation(
                        e_sb[:, :], t_g[:, :], AF.Exp,
                        bias=nmax[:, :], scale=1.0, accum_out=esum[:, :],
                    )
                    rec = p_stat.tile([128, 1], F32, tag="rec",
                                      name=f"rec{b}_{m}")
                    nc.vector.reciprocal(rec[:, :], esum[:, :])
                    sc = p_stat.tile([128, 1], F32, tag="sc",
                                     name=f"sc{b}_{m}")
                    nc.vector.tensor_tensor(
                        sc[:, :], rec[:, :], gamb[:, :], op=AluOpType.mult,
                    )
                    b_sb = p_b.tile([128, c], BF16, tag="bsb",
                                    name=f"bmat{b}_{m}")
                    nc.vector.scalar_tensor_tensor(
                        b_sb[:, :], e_sb[:, :], sc[:, :],
                        ident_rows[m][:, :],
                        op0=AluOpType.mult, op1=AluOpType.add,
                    )
                    b_rows.append(b_sb)

                for m in range(nm):
                    lo = m * 128
                    t_g = p_g.tile([128, c], BF16, tag="gsb", name=f"g{b}_{m}")
                    eng = nc.vector.tensor_copy if m % 2 else nc.scalar.copy
                    eng(t_g[:, lo:c], psg_rows[m][:, lo:c])
                    g_sb.append(t_g)
                    # mirror blocks (d, m) -> (m, d) for d < m
                    for d in range(m):
                        tp = p_pst.tile([128, 2, c], BF16, tag="pst",
                                        name=f"gt{b}_{m}_{d}")
                        nc.tensor.transpose(
                            tp[:, 0, 0:128],
                            g_sb[d][:, m * 128:(m + 1) * 128],
                            ident128,
                        )
                        nc.vector.tensor_copy(
                            t_g[:, d * 128:(d + 1) * 128], tp[:, 0, 0:128])
                    softmax_row(m)
                return b_rows

            def mm2_group(b, k, ft, b_rows):
                """Second matmul + store for one 256-row group."""
                o_sb = p_out.tile([128, 2, c], BF16, tag="outp",
                                  name=f"o{b}_{k}")
                for j in range(2):
                    ps_o = p_pso.tile([128, c], F32, tag="pso",
                                      name=f"pso{b}_{k}_{j}")
                    for m in range(nm):
                        nc.tensor.matmul(
                            ps_o[:, :],
                            ft[:, m, k * 256 + j * 128:
                               k * 256 + (j + 1) * 128],
                            b_rows[m][:, :],
                            start=(m == 0),
                            stop=(m == nm - 1),
                        )
                    cp_engine()(o_sb[:, j, :], ps_o[:, :])
                base = b * n + k * 256
                nc.sync.dma_start(
                    out=y_d[base:base + 256, :]
                        .rearrange("(p j) c1 -> p j c1", p=128),
                    in_=o_sb[:, :, :],
                )

            def body(_iv=None):
                # ---- batch 0 stream ----
                ft0 = p_ft.tile([128, nm, n], BF16, tag="ft", name="ft0")
                psg0 = [p_psg.tile([128, c], F32, tag=f"psg{m}",
                                   name=f"psg0_{m}") for m in range(nm)]
                fb0 = [load_chunk(0, k) for k in range(nkc)]
                if ablate == "loads":
                    for k in range(nkc):
                        # keep a reader so tiles are consumed
                        f8k = p_f8.tile([128, 2, c], FP8, tag="f8c",
                                        name=f"f8d0_{k}")
                        nc.vector.tensor_copy(f8k[:, :, :], fb0[k][:, :, :])
                    return
                # batch 1 loads queue right behind batch 0's: the SWDGE queue
                # runs continuously at full HBM rate while compute trails.
                fb1 = [load_chunk(1, k) for k in range(nkc)]
                for k in range(nkc):
                    chunk_work(0, k, fb0[k], ft0, psg0)
                if ablate in ("nogram", "nofinish"):
                    return
                b_rows0 = gram_finish(0, psg0)
                if ablate == "gram":
                    return

                # ---- batch 1 stream interleaved with batch 0 MM2 ----
                ft1 = p_ft.tile([128, nm, n], BF16, tag="ft", name="ft1")
                psg1 = [p_psg.tile([128, c], F32, tag=f"psg{m}",
                                   name=f"psg1_{m}") for m in range(nm)]
                # batch 1 gram parts are front-loaded 2-per-unit so gram 1
                # finishes ~when its last chunk lands; its softmax chain then
                # hides under the deferred second half of batch 0's MM2.
                for k in range(nkc // 2):
                    gram_part(1, 2 * k, fb1[2 * k], psg1)
                    gram_part(1, 2 * k + 1, fb1[2 * k + 1], psg1)
                    mm2_group(0, k, ft0, b_rows0)
                    tpose_part(1, 2 * k, fb1[2 * k], ft1)
                    tpose_part(1, 2 * k + 1, fb1[2 * k + 1], ft1)
                if ablate == "phase4":
                    return
                b_rows1 = gram_finish(1, psg1)
                for k in range(nkc // 2, nkc):
                    mm2_group(0, k, ft0, b_rows0)
                if ablate == "fin1":
                    return
                for k in range(nkc):
                    mm2_group(1, k, ft1, b_rows1)

            if reps is None:
                body()
            else:
                with tc.For_i(0, reps, 1,
                              staggered_reset=staggered,
                              hint_engines=(mybir.EngineType.PE,
                                            mybir.EngineType.DVE,
                                            mybir.EngineType.Activation,
                                            mybir.EngineType.Pool,
                                            mybir.EngineType.SP)) as iv:
                    body(iv)

    nc.compile()
    return nc


_NC_CACHE = {}


def _get_nc():
    if "full" not in _NC_CACHE:
        _NC_CACHE["full"] = build_nc()
    return _NC_CACHE["full"]


def make_in_maps(inputs_np, gamma_np):
    """Shard full inputs into per-core in_maps."""
    x = np.ascontiguousarray(
        np.asarray(inputs_np, dtype=np.float32).reshape(B_FULL, N, C)
    )
    gam = np.asarray(gamma_np, dtype=np.float32).reshape(1, 1)
    ident = np.eye(C, dtype=np.float32).astype(ml_dtypes.bfloat16)
    in_maps = []
    for core in range(N_CORES):
        xs = x[core * B_LOC:(core + 1) * B_LOC].reshape(B_LOC * N, C)
        in_maps.append({
            "x": np.ascontiguousarray(xs),
            "gamma": gam,
            "ident": ident,
        })
    return in_maps


def kernel(inputs, gamma):
    nc = _get_nc()
    in_maps = make_in_maps(inputs, gamma)
    res = run_bass_kernel_spmd(nc, in_maps, core_ids=list(range(N_CORES)))
    outs = [np.asarray(res.results[c]["y"], dtype=np.float32)
            .reshape(B_LOC, N, C) for c in range(N_CORES)]
    y = np.concatenate(outs, axis=0).reshape(B_FULL, H, W, C)
    return y.astype(np.float32)


# revision 32
# speedup vs baseline: 1.2388x; 1.0112x over previous
"""Trainium2 Bass kernel for nn_CAM (channel attention module).

Reference (per batch b):
    f = x[b].reshape(N, C)                      # N = H*W = 4096, C = 512
    G = f^T f                                   # (C, C) channel gram
    A = softmax(G, axis=-1)
    out[b] = gamma * (f @ A) + x[b]

Algebraic folds:
  * residual: out[b] = f @ (gamma * A + I) -- residual add folded into the
    second matmul's moving operand.
  * symmetry: G == G^T, so only upper-triangular 128-blocks are computed
    (free dims 512/384/256/128); the 6 lower blocks are PE-transposed back.

Layout: n rows are interleaved 2-per-partition (row 256k + 2p + j lives on
partition p, slice j of chunk k).  This makes load descriptors 4KB and store
descriptors 2KB contiguous (vs 2KB/1KB non-interleaved), which measured
~25% faster stores under full 8-core HBM contention.  The gram is invariant
to the n-permutation; ft/MM2/store all use the same ordering consistently.

Schedule (per core, 2 batches, streaming):
  - chunk k of batch b arrives (SWDGE cast fp32->bf16, 16 chunks/batch)
  - DVE casts it to fp8; PE transposes it into ft (f^T) and accumulates the
    triangular gram into 4 parallel PSUM banks (fp8 DoubleRow, 256-row
    contraction per chunk)
  - after the last chunk: G rows copy to SBUF (bf16), lower blocks are
    mirrored by 6 PE transposes, then per-row softmax (DVE max / ACT exp
    with accum / DVE reciprocal+scale) produces B = gamma/s * E + I (bf16)
  - MM2: out rows = ft_chunk^T @ B accumulated over the 4 channel blocks in
    PSUM; batch 1's loads queue immediately behind batch 0's so DMA runs
    continuously, and batch 0's MM2 groups interleave with batch 1's
    stream work on the PE.  Batch 1's gram matmuls are front-loaded so its
    softmax chain hides under the deferred second half of batch 0's MM2.
  - PSUM->SBUF copies are split ACT:DVE ~2:1; stores are HWDGE (sync)
    with one 2KB-descriptor store per 256-row group.

Sharding: pure data-parallel over batch: 16 batches -> 8 cores x 2.
"""

import sys

if "/opt/trn_rl_repo" not in sys.path:
    sys.path.insert(0, "/opt/trn_rl_repo")

import numpy as np
import ml_dtypes

import concourse.bacc as bacc
import concourse.mybir as mybir
import concourse.tile as tile
from concourse.alu_op_type import AluOpType
from concourse.bass_utils import run_bass_kernel_spmd

F32 = mybir.dt.float32
BF16 = mybir.dt.bfloat16
FP8 = mybir.dt.float8e4
AF = mybir.ActivationFunctionType

N_CORES = 8
B_FULL, H, W, C = 16, 64, 64, 512
N = H * W                      # 4096 spatial positions per batch
B_LOC = B_FULL // N_CORES      # 2 batches per core
NM = C // 128                  # 4 channel blocks
NKC = N // 256                 # 16 interleaved 256-row chunks per batch


def build_nc(b_loc=B_LOC, n=N, c=C, num_devices=N_CORES, reps=None,
             ablate=None, staggered=True, act_share=3, ft_flat=True,
             **_legacy):
    """Build + compile the per-core Bass program.

    reps: if set, wrap the body in a hardware For_i loop (timing builds).
    act_share: of every act_share psum->sbuf copies, 1 goes to DVE, the
        rest to ACT.
    """
    nkc = n // 256   # interleaved 256-row chunks
    nm = c // 128

    nc = bacc.Bacc(
        "TRN2",
        target_bir_lowering=False,
        debug=False,
        num_devices=num_devices,
    )

    x_d = nc.dram_tensor("x", [b_loc * n, c], F32, kind="ExternalInput")
    gam_d = nc.dram_tensor("gamma", [1, 1], F32, kind="ExternalInput")
    id_d = nc.dram_tensor("ident", [c, c], BF16, kind="ExternalInput")
    y_d = nc.dram_tensor("y", [b_loc * n, c], BF16, kind="ExternalOutput")

    with tile.TileContext(nc) as tc:
        with (
            tc.tile_pool(name="fbc", bufs=11) as p_fb,     # bf16 chunk staging
            tc.tile_pool(name="f8c", bufs=8) as p_f8,      # fp8 chunk staging
            tc.tile_pool(name="ft", bufs=2) as p_ft,       # f^T bf16 per batch
            tc.tile_pool(name="gsb", bufs=2 * nm) as p_g,
            tc.tile_pool(name="esb", bufs=2 * nm) as p_e,
            tc.tile_pool(name="bsb", bufs=2 * nm) as p_b,
            tc.tile_pool(name="stat", bufs=8 * nm) as p_stat,
            tc.tile_pool(name="outp", bufs=6) as p_out,
            tc.tile_pool(name="const", bufs=1) as p_const,
            tc.tile_pool(name="psg", bufs=1, space="PSUM") as p_psg,
            tc.tile_pool(name="pst", bufs=2, space="PSUM") as p_pst,
            tc.tile_pool(name="pso", bufs=2, space="PSUM") as p_pso,
        ):
            # --- constants (outside the timing loop) ---
            ident_rows = []
            for m in range(nm):
                t = p_const.tile([128, c], BF16, tag=f"ident{m}",
                                 name=f"ident{m}")
                nc.sync.dma_start(out=t[:, :],
                                  in_=id_d[m * 128:(m + 1) * 128, :])
                ident_rows.append(t)
            ident128 = ident_rows[0][:, 0:128]
            gam1 = p_const.tile([1, 1], F32, tag="gam1", name="gam1")
            nc.sync.dma_start(out=gam1[:, :], in_=gam_d[:, :])
            gamb = p_const.tile([128, 1], F32, tag="gamb", name="gamb")
            nc.gpsimd.partition_broadcast(gamb[:, :], gam1[:, :])

            copy_ctr = [0]

            def cp_engine():
                """Rotate psum->sbuf copies: 1 in act_share goes to DVE."""
                i = copy_ctr[0]
                copy_ctr[0] += 1
                if i % act_share == 0:
                    return nc.vector.tensor_copy
                return nc.scalar.copy

            def load_chunk(b, k):
                fbk = p_fb.tile([128, 2, c], BF16, tag="fbc",
                                name=f"fb{b}_{k}")
                base = b * n + k * 256
                nc.gpsimd.dma_start(
                    out=fbk[:, :, :],
                    in_=x_d[base:base + 256, :]
                        .rearrange("(p j) c1 -> p j c1", p=128),
                )
                return fbk

            def gram_part(b, k, fbk, psg_rows):
                """f8 cast + gram accumulation for chunk k."""
                f8k = p_f8.tile([128, 2, c], FP8, tag="f8c",
                                name=f"f8{b}_{k}")
                nc.vector.tensor_copy(f8k[:, :, :], fbk[:, :, :])
                # gram: one DoubleRow matmul per row-block (256-contraction)
                for m in range(nm if ablate != "nogram" else 0):
                    lo = m * 128
                    nc.tensor.matmul(
                        psg_rows[m][:, lo:c],
                        f8k[:, :, m * 128:(m + 1) * 128],
                        f8k[:, :, lo:c],
                        start=(k == 0),
                        stop=(k == nkc - 1),
                        perf_mode=mybir.MatmulPerfMode.DoubleRow,
                    )

            def tpose_part(b, k, fbk, ft):
                """8 transposes per chunk into one staging tile, one copy."""
                ps_t = p_pst.tile([128, 2, c], BF16, tag="pst",
                                  name=f"pst{b}_{k}")
                for j in range(2):
                    for m in range(nm):
                        nc.tensor.transpose(
                            ps_t[:, j, m * 128:(m + 1) * 128],
                            fbk[:, j, m * 128:(m + 1) * 128],
                            ident128,
                        )
                if ft_flat:
                    cp_engine()(ft[:, k, :, :], ps_t[:, :, :])
                else:
                    cp_engine()(
                        ft[:, :, k * 256: (k + 1) * 256]
                            .rearrange("p m (j f) -> p j m f", j=2),
                        ps_t[:, :, :].rearrange("p j (m f) -> p j m f", m=nm),
                    )

            def chunk_work(b, k, fbk, ft, psg_rows):
                gram_part(b, k, fbk, psg_rows)
                tpose_part(b, k, fbk, ft)

            def gram_finish(b, psg_rows):
                """PSUM G -> SBUF (bf16), mirror lower blocks, softmax -> B.

                Progressive: row m's softmax is emitted as soon as its copy
                and mirror blocks are available, so b_rows[0] (and MM2) can
                start while later rows are still being assembled.
                """
                g_sb = []
                b_rows = []

                def softmax_row(m):
                    t_g = g_sb[m]
                    nmax = p_stat.tile([128, 1], F32, tag="nmax",
                                       name=f"nmax{b}_{m}")
                    nc.vector.reduce_max(
                        nmax[:, :], t_g[:, :], axis=mybir.AxisListType.X,
                        negate=True,
                    )
                    e_sb = p_e.tile([128, c], BF16, tag="esb",
                                    name=f"e{b}_{m}")
                    esum = p_stat.tile([128, 1], F32, tag="esum",
                                       name=f"esum{b}_{m}")
                    nc.scalar.activation(
                        e_sb[:, :], t_g[:, :], AF.Exp,
                        bias=nmax[:, :], scale=1.0, accum_out=esum[:, :],
                    )
                    rec = p_stat.tile([128, 1], F32, tag="rec",
                                      name=f"rec{b}_{m}")
                    nc.vector.reciprocal(rec[:, :], esum[:, :])
                    sc = p_stat.tile([128, 1], F32, tag="sc",
                                     name=f"sc{b}_{m}")
                    nc.vector.tensor_tensor(
                        sc[:, :], rec[:, :], gamb[:, :], op=AluOpType.mult,
                    )
                    b_sb = p_b.tile([128, c], BF16, tag="bsb",
                                    name=f"bmat{b}_{m}")
                    nc.vector.scalar_tensor_tensor(
                        b_sb[:, :], e_sb[:, :], sc[:, :],
                        ident_rows[m][:, :],
                        op0=AluOpType.mult, op1=AluOpType.add,
                    )
                    b_rows.append(b_sb)

                for m in range(nm):
                    lo = m * 128
                    t_g = p_g.tile([128, c], BF16, tag="gsb", name=f"g{b}_{m}")
                    eng = nc.vector.tensor_copy if m % 2 else nc.scalar.copy
                    eng(t_g[:, lo:c], psg_rows[m][:, lo:c])
                    g_sb.append(t_g)
                    # mirror blocks (d, m) -> (m, d) for d < m
                    for d in range(m):
                        tp = p_pst.tile([128, 2, c], BF16, tag="pst",
                                        name=f"gt{b}_{m}_{d}")
                        nc.tensor.transpose(
                            tp[:, 0, 0:128],
                            g_sb[d][:, m * 128:(m + 1) * 128],
                            ident128,
                        )
                        nc.vector.tensor_copy(
                            t_g[:, d * 128:(d + 1) * 128], tp[:, 0, 0:128])
                    softmax_row(m)
                return b_rows

            def mm2_group(b, k, ft, b_rows):
                """Second matmul + store for one 256-row group."""
                o_sb = p_out.tile([128, 2, c], BF16, tag="outp",
                                  name=f"o{b}_{k}")
                for j in range(2):
                    ps_o = p_pso.tile([128, c], F32, tag="pso",
                                      name=f"pso{b}_{k}_{j}")
                    for m in range(nm):
                        stat = (ft[:, k, j, m * 128:(m + 1) * 128] if ft_flat
                                else ft[:, m, k * 256 + j * 128:
                                        k * 256 + (j + 1) * 128])
                        nc.tensor.matmul(
                            ps_o[:, :],
                            stat,
                            b_rows[m][:, :],
                            start=(m == 0),
                            stop=(m == nm - 1),
                        )
                    cp_engine()(o_sb[:, j, :], ps_o[:, :])
                base = b * n + k * 256
                nc.sync.dma_start(
                    out=y_d[base:base + 256, :]
                        .rearrange("(p j) c1 -> p j c1", p=128),
                    in_=o_sb[:, :, :],
                )

            def body(_iv=None):
                # ---- batch 0 stream ----
                ftshape = ([128, nkc, 2, nm * 128] if ft_flat
                           else [128, nm, n])
                ft0 = p_ft.tile(ftshape, BF16, tag="ft", name="ft0")
                psg0 = [p_psg.tile([128, c], F32, tag=f"psg{m}",
                                   name=f"psg0_{m}") for m in range(nm)]
                fb0 = [load_chunk(0, k) for k in range(nkc)]
                if ablate == "loads":
                    for k in range(nkc):
                        # keep a reader so tiles are consumed
                        f8k = p_f8.tile([128, 2, c], FP8, tag="f8c",
                                        name=f"f8d0_{k}")
                        nc.vector.tensor_copy(f8k[:, :, :], fb0[k][:, :, :])
                    return
                # batch 1 loads queue right behind batch 0's: the SWDGE queue
                # runs continuously at full HBM rate while compute trails.
                fb1 = [load_chunk(1, k) for k in range(nkc)]
                for k in range(nkc):
                    chunk_work(0, k, fb0[k], ft0, psg0)
                if ablate in ("nogram", "nofinish"):
                    return
                b_rows0 = gram_finish(0, psg0)
                if ablate == "gram":
                    return

                # ---- batch 1 stream interleaved with batch 0 MM2 ----
                ft1 = p_ft.tile(ftshape, BF16, tag="ft", name="ft1")
                psg1 = [p_psg.tile([128, c], F32, tag=f"psg{m}",
                                   name=f"psg1_{m}") for m in range(nm)]
                # batch 1 gram parts are front-loaded 2-per-unit so gram 1
                # finishes ~when its last chunk lands; its softmax chain then
                # hides under the deferred second half of batch 0's MM2.
                for k in range(nkc // 2):
                    gram_part(1, 2 * k, fb1[2 * k], psg1)
                    gram_part(1, 2 * k + 1, fb1[2 * k + 1], psg1)
                    mm2_group(0, k, ft0, b_rows0)
                    tpose_part(1, 2 * k, fb1[2 * k], ft1)
                    tpose_part(1, 2 * k + 1, fb1[2 * k + 1], ft1)
                if ablate == "phase4":
                    return
                b_rows1 = gram_finish(1, psg1)
                for k in range(nkc // 2, nkc):
                    mm2_group(0, k, ft0, b_rows0)
                if ablate == "fin1":
                    return
                for k in range(nkc):
                    mm2_group(1, k, ft1, b_rows1)

            if reps is None:
                body()
            else:
                with tc.For_i(0, reps, 1,
                              staggered_reset=staggered,
                              hint_engines=(mybir.EngineType.PE,
                                            mybir.EngineType.DVE,
                                            mybir.EngineType.Activation,
                                            mybir.EngineType.Pool,
                                            mybir.EngineType.SP)) as iv:
                    body(iv)

    nc.compile()
    return nc


_NC_CACHE = {}


def _get_nc():
    if "full" not in _NC_CACHE:
        _NC_CACHE["full"] = build_nc()
    return _NC_CACHE["full"]


def make_in_maps(inputs_np, gamma_np):
    """Shard full inputs into per-core in_maps."""
    x = np.ascontiguousarray(
        np.asarray(inputs_np, dtype=np.float32).reshape(B_FULL, N, C)
    )
    gam = np.asarray(gamma_np, dtype=np.float32).reshape(1, 1)
    ident = np.eye(C, dtype=np.float32).astype(ml_dtypes.bfloat16)
    in_maps = []
    for core in range(N_CORES):
        xs = x[core * B_LOC:(core + 1) * B_LOC].reshape(B_LOC * N, C)
        in_maps.append({
            "x": np.ascontiguousarray(xs),
            "gamma": gam,
            "ident": ident,
        })
    return in_maps


def kernel(inputs, gamma):
    nc = _get_nc()
    in_maps = make_in_maps(inputs, gamma)
    res = run_bass_kernel_spmd(nc, in_maps, core_ids=list(range(N_CORES)))
    outs = [np.asarray(res.results[c]["y"], dtype=np.float32)
            .reshape(B_LOC, N, C) for c in range(N_CORES)]
    y = np.concatenate(outs, axis=0).reshape(B_FULL, H, W, C)
    return y.astype(np.float32)
